# revision 1
# baseline (speedup 1.0000x reference)
"""Trainium2 Bass kernel for nn_Actor (GRU-over-vehicles + MLP head).

Strategy: pure data parallelism. B=16384 rows are split across 8 NeuronCores
(2048 rows each); the ~2M parameters are replicated. No collectives.

On-chip layout is "gate major": activations live as [feature, batch] tiles so
every matmul uses the (static) transposed weight as the stationary operand and
streams batch columns. All matmul inputs are bf16 (host-prepared), PSUM
accumulation is fp32. Host pre-transposes/pads all tensors so the kernel does
zero on-chip transposes.

Per (step t, batch group g of 512):
  PE   : r/z gates = W_ihT_rz x_t (+) W_hhT_rz h   (3 mm per 128-row chunk)
         xn = W_ihT_n x_t ; hn = W_hhT_n h
  ACT  : r = sigmoid(psum + b_r) ; z = sigmoid(psum + b_z) ; n = tanh(w + b_ihn)
  DVE  : u = (hn + b_hhn) * r ; w = u + xn ; a = z*d ; h' = n + a
  POOL : d = h - n
MLP head afterwards, group by group, weights stationary, ReLU+bias on ACT.
"""

import numpy as np
import ml_dtypes

import concourse.bass as bass
import concourse.tile as tile
from concourse import bacc
from concourse import mybir
from concourse.bass_utils import run_bass_kernel_spmd

BF16 = mybir.dt.bfloat16
F32 = mybir.dt.float32
Act = mybir.ActivationFunctionType
Alu = mybir.AluOpType

B, V, F, H = 16384, 20, 15, 256
NCORES = 8
BL = B // NCORES          # 2048 batch rows per core
GN = 512                  # batch-group width (PSUM bank = 512 fp32)
NG = BL // GN             # 4 groups
VS = 32                   # vehicle slot height in xT (15 real + 17 zero pad)

_NC_CACHE = {}


def _build_nc():
    nc = bacc.Bacc("TRN2", target_bir_lowering=False, debug=False)

    xT_d = nc.dram_tensor("xT", [V, 96, BL], BF16, kind="ExternalInput")
    # replicated 4x vertically so lhsT base partition matches the vehicle slot
    wih_d = nc.dram_tensor("w_ihT", [128, 768], BF16, kind="ExternalInput")
    whh_d = nc.dram_tensor("w_hhT", [2, 128, 768], BF16, kind="ExternalInput")
    w1x_d = nc.dram_tensor("w1T_x", [VS, 1024], BF16, kind="ExternalInput")
    w1h_d = nc.dram_tensor("w1T_h", [2, 128, 1024], BF16, kind="ExternalInput")
    w2_d = nc.dram_tensor("w2T", [8, 128, 1024], BF16, kind="ExternalInput")
    w3_d = nc.dram_tensor("w3T", [8, 128, 512], BF16, kind="ExternalInput")
    w4_d = nc.dram_tensor("w4T", [4, 128, 256], BF16, kind="ExternalInput")
    wp_d = nc.dram_tensor("wpT", [2, 128, 1], BF16, kind="ExternalInput")
    br_d = nc.dram_tensor("b_r", [128, 2], F32, kind="ExternalInput")
    bz_d = nc.dram_tensor("b_z", [128, 2], F32, kind="ExternalInput")
    bihn_d = nc.dram_tensor("b_ihn", [128, 2], F32, kind="ExternalInput")
    bhhn_d = nc.dram_tensor("b_hhn", [128, 2], F32, kind="ExternalInput")
    b1_d = nc.dram_tensor("b1", [128, 8], F32, kind="ExternalInput")
    b2_d = nc.dram_tensor("b2", [128, 8], F32, kind="ExternalInput")
    b3_d = nc.dram_tensor("b3", [128, 4], F32, kind="ExternalInput")
    b4_d = nc.dram_tensor("b4", [128, 2], F32, kind="ExternalInput")
    bp_d = nc.dram_tensor("bp", [1, 1], F32, kind="ExternalInput")
    out_d = nc.dram_tensor("out", [1, BL], F32, kind="ExternalOutput")

    with tile.TileContext(nc) as tc:
        with (
            tc.tile_pool(name="const", bufs=1) as consts,
            tc.tile_pool(name="psum", bufs=8, space=bass.MemorySpace.PSUM) as psum,
            tc.tile_pool(name="work", bufs=8) as work,
            tc.tile_pool(name="mlp", bufs=12) as mlp,
        ):
            def load(dram_ap, shape, dtype, tag):
                t = consts.tile(shape, dtype, tag=tag, name=tag)
                nc.sync.dma_start(t[:], dram_ap)
                return t

            wih = load(wih_d[:], [128, 768], BF16, "wih")
            x0 = load(xT_d[0], [96, BL], BF16, "x0")  # persistent: MLP input
            xtiles = {}

            def xtile(t):
                if t not in xtiles:
                    xt_ = work.tile([96, BL], BF16, tag="xt", name="xt", bufs=4)
                    nc.sync.dma_start(xt_[:], xT_d[t])
                    xtiles[t] = xt_
                return xtiles[t]
            whh = [load(whh_d[i], [128, 768], BF16, f"whh{i}") for i in range(2)]
            br = load(br_d[:], [128, 2], F32, "br")
            bz = load(bz_d[:], [128, 2], F32, "bz")
            bihn = load(bihn_d[:], [128, 2], F32, "bihn")
            bhhn = load(bhhn_d[:], [128, 2], F32, "bhhn")
            w1x = load(w1x_d[:], [VS, 1024], BF16, "w1x")
            w1h = [load(w1h_d[i], [128, 1024], BF16, f"w1h{i}") for i in range(2)]
            w2 = [load(w2_d[i], [128, 1024], BF16, f"w2_{i}") for i in range(8)]
            w3 = [load(w3_d[i], [128, 512], BF16, f"w3_{i}") for i in range(8)]
            w4 = [load(w4_d[i], [128, 256], BF16, f"w4_{i}") for i in range(4)]
            wp = [load(wp_d[i], [128, 1], BF16, f"wp{i}") for i in range(2)]
            b1 = load(b1_d[:], [128, 8], F32, "b1")
            b2 = load(b2_d[:], [128, 8], F32, "b2")
            b3 = load(b3_d[:], [128, 4], F32, "b3")
            b4 = load(b4_d[:], [128, 2], F32, "b4")
            bp = load(bp_d[:], [1, 1], F32, "bp")

            # hidden state, per (chunk m, group g): [128, GN] bf16
            h_sb = [
                [consts.tile([128, GN], BF16, tag=f"h{m}_{g}", name=f"h{m}_{g}") for g in range(NG)]
                for m in range(2)
            ]
            oT = consts.tile([1, BL], F32, tag="oT", name="oT")

            mm = nc.tensor.matmul

            # ---------------- GRU over V=20 vehicle steps ----------------
            for t in range(V):
                xa = x0 if t == 0 else xtile(t)
                xtile(min(t + 2, V - 1))  # prefetch 2 steps ahead
                for g in range(NG):
                    gs = slice(GN * g, GN * (g + 1))

                    # x-contributions: 6 matmuls in 2 waves of 3 concurrent
                    # row-strip (K=32) matmuls; vehicle data replicated at
                    # partition strips 0/32/64.
                    p_rz = [psum.tile([128, GN], F32, tag="ps", name="ps")
                            for _ in range(4)]
                    p_xn = [psum.tile([128, GN], F32, tag="ps", name="ps")
                            for _ in range(2)]
                    for m in range(3):  # wave 1: rz chunks 0..2
                        st = slice(32 * m, 32 * (m + 1))
                        mm(p_rz[m][:], wih[st, 128 * m : 128 * (m + 1)], xa[st, gs],
                           start=True, stop=(t == 0))
                    for i, (pt, ws) in enumerate(
                        [(p_rz[3], slice(384, 512)),
                         (p_xn[0], slice(512, 640)),
                         (p_xn[1], slice(640, 768))]
                    ):  # wave 2
                        st = slice(32 * i, 32 * (i + 1))
                        mm(pt[:], wih[st, ws], xa[st, gs],
                           start=True, stop=(t == 0 or pt is not p_rz[3]))

                    # h-contributions
                    p_hn = []
                    if t > 0:
                        for m in range(4):
                            ws = slice(128 * m, 128 * (m + 1))
                            mm(p_rz[m][:], whh[0][:, ws], h_sb[0][g][:], start=False, stop=False)
                            mm(p_rz[m][:], whh[1][:, ws], h_sb[1][g][:], start=False, stop=True)
                        for m in range(2):
                            ws = slice(512 + 128 * m, 512 + 128 * (m + 1))
                            p = psum.tile([128, GN], F32, tag="ps", name="ps")
                            mm(p[:], whh[0][:, ws], h_sb[0][g][:], start=True, stop=False)
                            mm(p[:], whh[1][:, ws], h_sb[1][g][:], start=False, stop=True)
                            p_hn.append(p)

                    r_t, z_t = [], []
                    for m in range(2):
                        r_ = work.tile([128, GN], BF16, tag="r", name="r", bufs=6)
                        nc.scalar.activation(r_[:], p_rz[m][:], Act.Sigmoid, bias=br[:, m : m + 1])
                        r_t.append(r_)
                    for m in range(2):
                        z_ = work.tile([128, GN], BF16, tag="z", name="z", bufs=6)
                        nc.scalar.activation(z_[:], p_rz[2 + m][:], Act.Sigmoid, bias=bz[:, m : m + 1])
                        z_t.append(z_)

                    for m in range(2):
                        u_ = work.tile([128, GN], BF16, tag="u", name="u", bufs=6)
                        if t == 0:
                            nc.vector.tensor_scalar_mul(u_[:], r_t[m][:], bhhn[:, m : m + 1])
                        else:
                            nc.vector.scalar_tensor_tensor(
                                u_[:], p_hn[m][:], bhhn[:, m : m + 1], r_t[m][:],
                                Alu.add, Alu.mult,
                            )
                        w_ = work.tile([128, GN], BF16, tag="w", name="w", bufs=6)
                        nc.vector.tensor_add(w_[:], u_[:], p_xn[m][:])
                        n_ = work.tile([128, GN], BF16, tag="n", name="n", bufs=6)
                        nc.scalar.activation(n_[:], w_[:], Act.Tanh, bias=bihn[:, m : m + 1])
                        if t == 0:
                            a_ = work.tile([128, GN], BF16, tag="a", name="a", bufs=6)
                            nc.vector.tensor_mul(a_[:], z_t[m][:], n_[:])
                            nc.vector.tensor_sub(h_sb[m][g][:], n_[:], a_[:])
                        else:
                            d_ = work.tile([128, GN], BF16, tag="d", name="d", bufs=6)
                            nc.gpsimd.tensor_sub(d_[:], h_sb[m][g][:], n_[:])
                            a_ = work.tile([128, GN], BF16, tag="a", name="a", bufs=6)
                            nc.vector.tensor_mul(a_[:], z_t[m][:], d_[:])
                            nc.vector.tensor_add(h_sb[m][g][:], n_[:], a_[:])

            # ---------------- MLP head ----------------
            # No recurrence here, so (chunk, K)-major order with group-inner
            # sweep is safe: one weight load serves 4 consecutive matmuls.
            GSL = [slice(GN * g, GN * (g + 1)) for g in range(NG)]

            def mlp_layer(n_out_chunks, k_tiles, rhs_of, w_of, bias, out_tag, out_bufs):
                outs = []
                for m in range(n_out_chunks):
                    pq = [psum.tile([128, GN], F32, tag="ps", name="ps_mlp")
                          for _ in range(NG)]
                    for ki in range(k_tiles):
                        for g in range(NG):
                            mm(pq[g][:], w_of(ki, m), rhs_of(ki, g),
                               start=(ki == 0), stop=(ki == k_tiles - 1))
                    a_ = mlp.tile([128, BL], BF16, tag=out_tag, name=out_tag, bufs=out_bufs)
                    for g in range(NG):
                        nc.scalar.activation(a_[:, GSL[g]], pq[g][:], Act.Relu,
                                             bias=bias[:, m : m + 1])
                    outs.append(a_)
                return outs

            def l1_rhs(ki, g):
                if ki == 0:
                    return x0[0:VS, GSL[g]]
                return h_sb[ki - 1][g][:]

            def l1_w(ki, m):
                wt = w1x if ki == 0 else w1h[ki - 1]
                return wt[:, 128 * m : 128 * (m + 1)]

            a1 = mlp_layer(8, 3, l1_rhs, l1_w, b1, "a1", 8)
            a2 = mlp_layer(8, 8, lambda ki, g: a1[ki][:, GSL[g]],
                           lambda ki, m: w2[ki][:, 128 * m : 128 * (m + 1)], b2, "a2", 8)
            a3 = mlp_layer(4, 8, lambda ki, g: a2[ki][:, GSL[g]],
                           lambda ki, m: w3[ki][:, 128 * m : 128 * (m + 1)], b3, "a3", 4)
            a4 = mlp_layer(2, 4, lambda ki, g: a3[ki][:, GSL[g]],
                           lambda ki, m: w4[ki][:, 128 * m : 128 * (m + 1)], b4, "a4", 2)

            po = [psum.tile([1, GN], F32, tag="ps", name="ps_o") for _ in range(NG)]
            for ki in range(2):
                for g in range(NG):
                    mm(po[g][:], wp[ki][:, 0:1], a4[ki][:, GSL[g]],
                       start=(ki == 0), stop=(ki == 1))
            for g in range(NG):
                nc.scalar.activation(oT[0:1, GSL[g]], po[g][:], Act.Tanh, bias=bp[0:1, 0:1])

            nc.sync.dma_start(out_d[:], oT[:])

    nc.compile()
    return nc


def _get_nc():
    if "nc" not in _NC_CACHE:
        _NC_CACHE["nc"] = _build_nc()
    return _NC_CACHE["nc"]


def _pad_vs(a):
    """[15, N] -> [VS, N] zero-padded."""
    out = np.zeros((VS, a.shape[1]), dtype=a.dtype)
    out[: a.shape[0]] = a
    return out


def _prep_shared(inputs):
    f4 = np.float32
    bf = ml_dtypes.bfloat16

    def g(name):
        return np.asarray(inputs[name], dtype=f4)

    W_ih, W_hh = g("W_ih"), g("W_hh")
    b_ih, b_hh = g("b_ih"), g("b_hh")
    W1, W2, W3, W4, Wp = g("W1"), g("W2"), g("W3"), g("W4"), g("Wp")
    b1, b2, b3, b4, bp = g("b1"), g("b2"), g("b3"), g("b4"), g("bp")

    shared = {
        "w_ihT": np.ascontiguousarray(np.tile(_pad_vs(W_ih.T), (4, 1))).astype(bf),  # 4x replicated; slots use first 3
        "w_hhT": np.ascontiguousarray(W_hh.T.reshape(2, 128, 768)).astype(bf),
        "w1T_x": np.ascontiguousarray(_pad_vs(W1.T[:15])).astype(bf),
        "w1T_h": np.ascontiguousarray(W1.T[15:].reshape(2, 128, 1024)).astype(bf),
        "w2T": np.ascontiguousarray(W2.T.reshape(8, 128, 1024)).astype(bf),
        "w3T": np.ascontiguousarray(W3.T.reshape(8, 128, 512)).astype(bf),
        "w4T": np.ascontiguousarray(W4.T.reshape(4, 128, 256)).astype(bf),
        "wpT": np.ascontiguousarray(Wp.T.reshape(2, 128, 1)).astype(bf),
        "b_r": np.ascontiguousarray((b_ih[0:256] + b_hh[0:256]).reshape(2, 128).T),
        "b_z": np.ascontiguousarray((b_ih[256:512] + b_hh[256:512]).reshape(2, 128).T),
        "b_ihn": np.ascontiguousarray(b_ih[512:768].reshape(2, 128).T),
        "b_hhn": np.ascontiguousarray(b_hh[512:768].reshape(2, 128).T),
        "b1": np.ascontiguousarray(b1.reshape(8, 128).T),
        "b2": np.ascontiguousarray(b2.reshape(8, 128).T),
        "b3": np.ascontiguousarray(b3.reshape(4, 128).T),
        "b4": np.ascontiguousarray(b4.reshape(2, 128).T),
        "bp": bp.reshape(1, 1).astype(f4),
    }
    return shared


def _prep_xT(state_core):
    """state [BL, 20, 15] f32 -> [20, 96, BL] bf16: each vehicle's 15 feature
    rows replicated at partition strips 0/32/64 so three K=32 matmuls can run
    concurrently in different PE row groups."""
    s = state_core.transpose(1, 2, 0)  # [20, 15, BL]
    xp = np.zeros((V, 3, VS, BL), dtype=np.float32)
    xp[:, :, :F, :] = s[:, None, :, :]
    return xp.reshape(V, 96, BL).astype(ml_dtypes.bfloat16)


def run(inputs, trace=False):
    nc = _get_nc()
    shared = _prep_shared(inputs)
    state = np.asarray(inputs["state"], dtype=np.float32)
    in_maps = []
    for c in range(NCORES):
        m = dict(shared)
        m["xT"] = _prep_xT(state[BL * c : BL * (c + 1)])
        in_maps.append(m)
    res = run_bass_kernel_spmd(nc, in_maps, list(range(NCORES)), trace=trace)
    out = np.concatenate(
        [np.asarray(res.results[c]["out"]).reshape(BL) for c in range(NCORES)]
    )
    return out.reshape(B, 1).astype(np.float32), res


def kernel(**inputs):
    out, _ = run(inputs, trace=False)
    return out

